# revision 2
# baseline (speedup 1.0000x reference)
"""DiffAttention Trainium2 Bass kernel v4 (shipped) (8-core head-parallel SPMD).

v3 changes vs v2 (driven by NTFF hardware profiles):
  - software-pipelined attention inner loop: scores(jt+1) issue ahead of
    AV/L(jt) so the PE never head-of-line blocks on the exp activation
  - combine uses the identity attn = (au1 - lam*L1/L2*au2)/L1 with the
    1/L1 scale absorbed into the RMSNorm (eps shift ~eps/L1^2,
    negligible): one ACT-engine Reciprocal + one gpsimd broadcast per
    block instead of two slow DVE reciprocals + two broadcasts
  - deferred RMSNorm via ACT Rsqrt; subln_w * (1-lambda_init) folded
    into woT rows on the host; bf16 broadcast + in-place bf16 multiply
  - Q/K PSUM evictions moved from Scalar to Vector engine
  - merged two-stream AV matmul ([128,1024] moving) on full key tiles
  - single strided-descriptor DMAs for wT / xT-block / woT-block loads
  - woT pool opened during attention so its DMAs prefetch under D
"""

import numpy as np
from contextlib import ExitStack

import concourse.bass as bass
import concourse.bacc as bacc
import concourse.tile as tile
from concourse import mybir
from concourse.bass_utils import run_bass_kernel_spmd

F32 = mybir.dt.float32
BF16 = mybir.dt.bfloat16
AF = mybir.ActivationFunctionType
OP = mybir.AluOpType

B, S, DIM = 2, 2048, 2048
NH, HD, HHD = 16, 128, 64
NC = 8
HPC = NH // NC          # 2 heads per core
E = HPC * HD            # 256 projection rows per core
T = B * S               # 4096 flattened tokens
ND = DIM // 128         # 16 d-tiles
NTB = T // 512          # 8 t-blocks
LAMBDA_INIT = 0.2
EPS = 1e-5

_CACHE = {}


def _build_program(nrep=1):
    nc = bacc.Bacc("TRN2", target_bir_lowering=False, debug=False, num_devices=NC)

    xT_d = nc.dram_tensor("xT", [DIM, T], BF16, kind="ExternalInput").ap()
    wqT_d = nc.dram_tensor("wqT", [DIM, E], BF16, kind="ExternalInput").ap()
    wkT_d = nc.dram_tensor("wkT", [DIM, E], BF16, kind="ExternalInput").ap()
    wvT_d = nc.dram_tensor("wvT", [DIM, E], BF16, kind="ExternalInput").ap()
    woT_d = nc.dram_tensor("woT", [DIM, DIM], BF16, kind="ExternalInput").ap()
    cos_d = nc.dram_tensor("cosT", [128, S], BF16, kind="ExternalInput").ap()
    sin_d = nc.dram_tensor("sinT", [128, S], BF16, kind="ExternalInput").ap()
    mask_d = nc.dram_tensor("mask", [128, 128], BF16, kind="ExternalInput").ap()
    lam_d = nc.dram_tensor("lam", [1, 1], F32, kind="ExternalInput").ap()
    out_d = nc.dram_tensor("out", [B, E, DIM], F32, kind="ExternalOutput").ap()

    with tile.TileContext(nc) as tc:
        for rep in range(nrep):
            ctx = ExitStack()
            consts = ctx.enter_context(tc.tile_pool(name="consts", bufs=1))
            # projection weights resident: [128, (dt, feat)], one DMA each;
            # wq first so the very first matmul chain unblocks asap
            wT = {}
            for wname, wd in (("q", wqT_d), ("k", wkT_d), ("v", wvT_d)):
                wall = consts.tile([128, ND * E], BF16, tag=f"w{wname}T",
                                   name=f"w{wname}T")
                wT[wname] = wall
                nc.sync.dma_start(
                    out=wall[:].rearrange("p (dt e) -> p dt e", dt=ND),
                    in_=wd.rearrange("(dt p) e -> p dt e", p=128))
            mask_t = consts.tile([128, 128], BF16)
            nc.sync.dma_start(out=mask_t, in_=mask_d)
            ones_tmp = consts.tile([128, 1], BF16)
            nc.vector.memset(ones_tmp, 1.0)
            ones_col = consts.tile([128, 1], BF16)
            nc.scalar.copy(out=ones_col, in_=ones_tmp)
            lam_t = consts.tile([1, 1], F32)
            nc.sync.dma_start(out=lam_t, in_=lam_d)
            cos_t = consts.tile([128, S], BF16)
            nc.sync.dma_start(out=cos_t, in_=cos_d)
            sin_t = consts.tile([128, S], BF16)
            nc.sync.dma_start(out=sin_t, in_=sin_d)

            # persistent per-(head,batch) projections + attn outputs
            qkv = ctx.enter_context(tc.tile_pool(name="qkv", bufs=1, side="right"))
            qT_sb = {}
            kT_sb = {}
            v_sb = {}
            for b in range(B):
                vt = qkv.tile([128, HPC * S], BF16, tag=f"v{b}", name=f"v{b}")
                v_sb[b] = vt
                for hl in range(HPC):
                    qT_sb[(hl, b)] = qkv.tile([128, S], BF16, tag=f"q{hl}{b}",
                                              name=f"q{hl}{b}")
                    kT_sb[(hl, b)] = qkv.tile([128, S], BF16, tag=f"k{hl}{b}",
                                              name=f"k{hl}{b}")
            attnN_pool = ctx.enter_context(tc.tile_pool(name="attnN", bufs=1))
            attnN = {}
            msbuf = {}
            for b in range(B):
                for hl in range(HPC):
                    attnN[(b, hl)] = attnN_pool.tile(
                        [128, S], BF16, tag=f"attnN{b}_{hl}", name=f"attnN{b}_{hl}")
                    msbuf[(b, hl)] = attnN_pool.tile(
                        [1, S], F32, tag=f"msb{b}_{hl}", name=f"msb{b}_{hl}")

            b_ctx = ExitStack()
            swp = b_ctx.enter_context(tc.tile_pool(name="swp", bufs=1))
            ropes = b_ctx.enter_context(tc.tile_pool(name="ropes", bufs=1))
            xTpool = b_ctx.enter_context(tc.tile_pool(name="xT", bufs=2))
            psP = b_ctx.enter_context(tc.tile_pool(name="psP", bufs=3, space="PSUM"))
            sw = {}

            def phase_b_block(tb):
                # xT tile for this 512-token block: [128, (dt, 512)], one DMA
                bb, trel = divmod(tb, 4)
                xT = xTpool.tile([128, ND * 512], BF16, tag="xTa", name="xTa")
                nc.sync.dma_start(
                    out=xT[:].rearrange("p (dt t) -> p dt t", dt=ND),
                    in_=xT_d[:, tb * 512:(tb + 1) * 512].rearrange(
                        "(dt p) t -> p dt t", p=128))
                # Q^T, K^T feature-major: out[feat, t] += wT[d, feat] * xT[d, t]
                for wname, dst in (("q", qT_sb), ("k", kT_sb)):
                    for et in range(HPC):
                        pp = psP.tile([128, 512], F32, tag="qkp", name="qkp")
                        for dt in range(ND):
                            nc.tensor.matmul(
                                pp, wT[wname][:, dt * E + et * 128:dt * E + (et + 1) * 128],
                                xT[:, dt * 512:(dt + 1) * 512],
                                start=(dt == 0), stop=(dt == ND - 1))
                        nc.vector.tensor_copy(
                            out=dst[(et, bb)][:, trel * 512:(trel + 1) * 512],
                            in_=pp)
                # V token-major: out[t, feat] += xT[d, t] * wvT[d, feat]
                for ts in range(4):
                    pp = psP.tile([128, E], F32, tag="vp", name="vp")
                    for dt in range(ND):
                        nc.tensor.matmul(
                            pp, xT[:, dt * 512 + ts * 128:dt * 512 + (ts + 1) * 128],
                            wT["v"][:, dt * E:(dt + 1) * E],
                            start=(dt == 0), stop=(dt == ND - 1))
                    tt = trel * 4 + ts
                    nc.scalar.copy(
                        out=v_sb[bb][:].rearrange(
                            "p (h tt u) -> p h tt u", h=HPC, tt=S // 128)[:, :, tt, :],
                        in_=pp[:].rearrange("p (h u) -> p h u", h=HPC))

            def rope_block(b):
                # partition-swapped copies: rows [32:64,0:32,96:128,64:96]
                for hl in range(HPC):
                    for nm, src in (("q", qT_sb[(hl, b)]), ("k", kT_sb[(hl, b)])):
                        dst = swp.tile([128, S], BF16, tag=f"{nm}sw{hl}",
                                       name=f"{nm}sw{hl}")
                        sw[(nm, hl)] = dst
                        for blk in range(4):
                            sb0 = (blk ^ 1) * 32
                            nc.sync.dma_start(
                                out=dst[blk * 32:(blk + 1) * 32, :],
                                in_=src[sb0:sb0 + 32, :])
                # rope: t = t*cos + tsw*sinsgn (unscaled; 1/8 folded into exp)
                for hl in range(HPC):
                    for nm, t_ in (("q", qT_sb[(hl, b)]), ("k", kT_sb[(hl, b)])):
                        sw_ = sw[(nm, hl)]
                        m1 = ropes.tile([128, S], BF16, tag="m1", name="m1")
                        nc.vector.tensor_mul(m1, t_, cos_t)
                        nc.vector.tensor_mul(sw_, sw_, sin_t)
                        nc.vector.tensor_add(t_, m1, sw_)

            def attn_block(b, hl, expp, cmb, psS, psAU, psL):
                at_t = attnN[(b, hl)]
                msb = msbuf[(b, hl)]
                qr = qT_sb[(hl, b)]
                kr = kT_sb[(hl, b)]
                vh = v_sb[b][:, hl * S:(hl + 1) * S]
                for ib in range(4):
                    i0 = ib * 512
                    njt = ib * 4 + 4
                    au = psAU.tile([128, 1024], F32, tag="au", name="au")
                    Lap = [psL.tile([1, 512], F32, tag=f"L{s_}", name=f"L{s_}")
                           for s_ in range(2)]
                    prev = None

                    def issue_avl(jt, c0, ex):
                        j0 = jt * 128
                        for s_ in range(2):
                            o0 = s_ * 512
                            nc.tensor.matmul(
                                au[:, o0 + c0:o0 + 512],
                                vh[:, j0:j0 + 128], ex[:, o0 + c0:o0 + 512],
                                start=(jt == 0), stop=(jt == njt - 1),
                                skip_group_check=True)
                        for s_ in range(2):
                            o0 = s_ * 512
                            nc.tensor.matmul(
                                Lap[s_][:, c0:512], ones_col,
                                ex[:, o0 + c0:o0 + 512],
                                start=(jt == 0), stop=(jt == njt - 1),
                                skip_group_check=True)

                    for jt in range(njt):
                        j0 = jt * 128
                        r = jt - ib * 4
                        c0 = max(r, 0) * 128  # cols left of this are fully
                        # masked for diagonal tiles: skip them
                        sp = psS.tile([128, 1024], F32, tag="sp", name="sp")
                        ex = expp.tile([128, 1024], BF16, tag="ex", name="ex")
                        for s_ in range(2):
                            e0 = s_ * 64
                            o0 = s_ * 512
                            nc.tensor.matmul(
                                sp[:, o0 + c0:o0 + 512],
                                kr[e0:e0 + 64, j0:j0 + 128],
                                qr[e0:e0 + 64, i0 + c0:i0 + 512])
                        if c0 == 0:
                            nc.scalar.activation(out=ex, in_=sp, func=AF.Exp,
                                                 scale=0.125)
                        else:
                            nc.scalar.activation(
                                out=ex[:].rearrange(
                                    "p (s c) -> p s c", s=2)[:, :, c0:512],
                                in_=sp[:].rearrange(
                                    "p (s c) -> p s c", s=2)[:, :, c0:512],
                                func=AF.Exp, scale=0.125)
                        if r >= 0:
                            m0 = r * 128  # triangle block
                            for s_ in range(2):
                                o0 = s_ * 512
                                nc.vector.tensor_mul(
                                    ex[:, o0 + m0:o0 + m0 + 128],
                                    ex[:, o0 + m0:o0 + m0 + 128], mask_t)
                        if prev is not None:
                            issue_avl(*prev)
                        prev = (jt, c0, ex)
                    issue_avl(*prev)
                    # combine: w = au0 - (lam*L1/L2) * au1; RMSNorm absorbs 1/L1
                    r1 = cmb.tile([1, 512], F32, tag="r1", name="r1")
                    nc.vector.tensor_scalar_mul(r1, Lap[0][:], lam_t[:])
                    r2 = cmb.tile([1, 512], F32, tag="r2", name="r2")
                    nc.vector.reciprocal(r2, Lap[1][:])
                    rr = cmb.tile([1, 512], F32, tag="rr", name="rr")
                    nc.vector.tensor_mul(rr, r1, r2)
                    brr = cmb.tile([128, 512], F32, tag="brr", name="brr")
                    nc.gpsimd.partition_broadcast(brr, rr)
                    t2 = cmb.tile([128, 512], F32, tag="t2", name="t2")
                    nc.vector.tensor_mul(t2, au[:, 512:1024], brr)
                    at = at_t[:, i0:i0 + 512]
                    nc.vector.tensor_sub(at, au[:, 0:512], t2)
                    sq = cmb.tile([128, 512], BF16, tag="sq", name="sq")
                    nc.vector.tensor_mul(sq, at, at)
                    msp = psL.tile([1, 512], F32, tag="L0", name="msp")
                    nc.tensor.matmul(msp, ones_col, sq, skip_group_check=True)
                    # msbuf <- ms/128 + eps (one fused DVE op)
                    nc.vector.tensor_scalar(
                        msb[:, i0:i0 + 512], msp, 1.0 / 128.0, EPS,
                        OP.mult, OP.add)

            # ---- schedule: B(b0) -> rope(b0) || B(b1) -> rope(b1) || D ----
            for tb in range(4):
                phase_b_block(tb)
            rope_block(0)
            for tb in range(4, 8):
                phase_b_block(tb)
            rope_block(1)
            b_ctx.close()
            woT_ctx = ExitStack()
            woTpool = woT_ctx.enter_context(tc.tile_pool(name="woT", bufs=2))
            d_pools = ExitStack()
            expp = d_pools.enter_context(tc.tile_pool(name="expp", bufs=3))
            cmb = d_pools.enter_context(tc.tile_pool(name="cmb", bufs=1))
            dnorm = d_pools.enter_context(tc.tile_pool(name="dnorm", bufs=1))
            psS = d_pools.enter_context(tc.tile_pool(name="psS", bufs=2, space="PSUM"))
            psAU = d_pools.enter_context(tc.tile_pool(name="psAU", bufs=1, space="PSUM"))
            psL = d_pools.enter_context(tc.tile_pool(name="psL", bufs=1, space="PSUM"))
            for b in range(B):
                for hl in range(HPC):
                    attn_block(b, hl, expp, cmb, psS, psAU, psL)
            for b in range(B):
                for hl in range(HPC):
                    nc.scalar.activation(out=msbuf[(b, hl)],
                                         in_=msbuf[(b, hl)], func=AF.Ln)
            for b in range(B):
                for hl in range(HPC):
                    rs = dnorm.tile([1, S], BF16, tag=f"rs{hl}",
                                    name=f"rs{hl}")
                    nc.scalar.activation(out=rs, in_=msbuf[(b, hl)],
                                         func=AF.Exp, scale=-0.5)
                    brs = dnorm.tile([128, S], BF16, tag="brs", name="brs",
                                     bufs=2)
                    nc.gpsimd.partition_broadcast(brs, rs)
                    nc.vector.tensor_mul(attnN[(b, hl)], attnN[(b, hl)], brs)
            d_pools.close()

            # ---------------- phase C: output projection ----------------
            with tc.tile_pool(name="psO", bufs=2, space="PSUM") as psO, \
                 tc.tile_pool(name="ostage", bufs=2) as ostage:
                for mb in range(4):
                    # woT tile: [128, (v16, 512)], one DMA
                    woTt = woTpool.tile([128, 16 * 512], BF16, tag="woTa",
                                        name="woTa")
                    nc.sync.dma_start(
                        out=woTt[:].rearrange("p (v n) -> p v n", v=16),
                        in_=woT_d[:, mb * 512:(mb + 1) * 512].rearrange(
                            "(v p) n -> p v n", p=128))
                    for b in range(B):
                        for hl in range(HPC):
                            y = attnN[(b, hl)][:].rearrange(
                                "p (u v) -> p v u", v=16)
                            op = psO.tile([128, 512], F32, tag="op", name="op")
                            for v16 in range(16):
                                nc.tensor.matmul(
                                    op, y[:, v16, :],
                                    woTt[:, v16 * 512:(v16 + 1) * 512],
                                    start=(v16 == 0), stop=(v16 == 15))
                            ost = ostage.tile([128, 512], F32, tag="ost",
                                              name="ost")
                            nc.scalar.copy(out=ost, in_=op)
                            nc.sync.dma_start(
                                out=out_d[b, hl * 128:(hl + 1) * 128,
                                          mb * 512:(mb + 1) * 512],
                                in_=ost)
            woT_ctx.close()

            ctx.close()

    nc.compile()
    return nc


def get_program(nrep=1):
    key = f"nc{nrep}"
    if key not in _CACHE:
        _CACHE[key] = _build_program(nrep)
    return _CACHE[key]


def _prep_in_maps(inputs):
    import ml_dtypes
    bf16 = ml_dtypes.bfloat16
    inp = {k: np.ascontiguousarray(np.asarray(v, dtype=np.float32))
           for k, v in inputs.items()}
    perm = np.concatenate([
        np.arange(0, 64, 2), np.arange(1, 64, 2),
        np.arange(64, 128, 2), np.arange(65, 128, 2)])
    wq_p = inp["wq"].reshape(NH, HD, DIM)[:, perm, :].reshape(NH * HD, DIM)
    wk_p = inp["wk"].reshape(NH, HD, DIM)[:, perm, :].reshape(NH * HD, DIM)

    fc = inp["freq_cis"]
    cosP = fc[:, :, 0, 0].T.astype(np.float32)
    sinP = fc[:, :, 1, 0].T.astype(np.float32)
    COS = np.concatenate([cosP[0:32], cosP[0:32], cosP[32:64], cosP[32:64]], 0)
    SIN = np.concatenate([-sinP[0:32], sinP[0:32], -sinP[32:64], sinP[32:64]], 0)

    mask = (np.arange(128)[:, None] <= np.arange(128)[None, :])
    lam1 = np.exp(np.sum(inp["lambda_q1"] * inp["lambda_k1"], dtype=np.float32))
    lam2 = np.exp(np.sum(inp["lambda_q2"] * inp["lambda_k2"], dtype=np.float32))
    lam = np.array([[lam1 - lam2 + LAMBDA_INIT]], np.float32)

    xT = np.ascontiguousarray(inp["x"].reshape(T, DIM).T.astype(bf16))
    # fold subln_w * (1 - lambda_init) into woT rows (row m scales feature
    # d = m % 128 of the attention output)
    subw = (inp["subln_w"] * (1.0 - LAMBDA_INIT)).astype(np.float32)
    woT = inp["wo"].T * np.tile(subw, DIM // HD)[:, None]
    woT = np.ascontiguousarray(woT.astype(bf16))
    common = {
        "xT": xT, "woT": woT,
        "cosT": np.ascontiguousarray(COS.astype(bf16)),
        "sinT": np.ascontiguousarray(SIN.astype(bf16)),
        "mask": np.ascontiguousarray(mask.astype(bf16)),
        "lam": lam,
    }
    in_maps = []
    for c in range(NC):
        m = dict(common)
        m["wqT"] = np.ascontiguousarray(wq_p[c * E:(c + 1) * E].T.astype(bf16))
        m["wkT"] = np.ascontiguousarray(wk_p[c * E:(c + 1) * E].T.astype(bf16))
        m["wvT"] = np.ascontiguousarray(
            inp["wv"][c * E:(c + 1) * E].T.astype(bf16))
        in_maps.append(m)
    return in_maps


def run(inputs, trace=False, nrep=1, **kw):
    nc = get_program(nrep)
    in_maps = _prep_in_maps(inputs)
    res = run_bass_kernel_spmd(nc, in_maps, list(range(NC)), trace=trace, **kw)
    out = np.empty((B, S, DIM), np.float32)
    for c in range(NC):
        out[:, c * E:(c + 1) * E, :] = res.results[c]["out"]
    return out, res


def kernel(**inputs):
    out, _ = run(inputs)
    return out
